# revision 35
# baseline (speedup 1.0000x reference)
# Causal multi-head attention block (QKV proj -> causal softmax attention -> out proj)
# for B=2, S=2048, C=1024, NH=16 on 8 Trainium2 NeuronCores.
#
# Sharding: core = b * 4 + head_group (data parallel on B, tensor parallel over
# 4 groups of 4 heads). Host pre-slices + transposes weights per core and sums
# the 4 partial out-projections per batch entry (row-parallel reduce) + Wo_b.
#
# Single fused pipeline over the 4 q-chunks (SC=512): attention(qc) runs with
# QKV-projection(qc+1) matmuls interleaved into the PE stream as fillers, so
# the PE never idles waiting on the Act engine's exp; all out-projections are
# deferred into the last (ACT-bound) chunk as its fillers.
# Scores stay f32r (K/Q from bf16 x/W via f32 psum); probs/V/attnT/Wo/out all
# bf16 (same 1 cycle/row on the PE, half the DMA + SBUF traffic).
# Softmax: no max-subtraction (unit-scale gaussian scores, exp safe in f32);
# denominator via an appended ones column on V; causal masking via column-
# range shrinking + a precomputed triangle mask multiply on DVE (variant
# "gps") or gpsimd affine_select (variant "pe").
# Normalize (per pair): phase A stages the AV psum to SBUF + fast reciprocal
# (frees the av bank for the next pair's start=True in ~1us); phase B
# (partition-broadcast + multiply into attnT) is deferred into the next
# pair's filler window. HW quirks honored: custom-DVE ops mis-read nonzero
# base partitions; mixing gpsimd op types costs ~5us ucode library reloads;
# HAM re-throttles the PE to 1.2 GHz after idle gaps (warmup matmul bursts
# bridge the DMA head and the final normalize->out-projection tail).

import numpy as np
import ml_dtypes

B, S, C, NH = 2, 2048, 1024, 16
HD = C // NH            # 64
NCORES = 8
GROUPS = 4              # head groups (cores per batch entry)
LH = NH // GROUPS       # 4 local heads per core
LC = LH * HD            # 256 local channels
SC = 512                # s-chunk (matmul moving free dim)
NSC = S // SC           # 4
KT = 128                # k tile
NKT = S // KT           # 16
CT = 128                # contraction tile
NCT = C // CT           # 8

_CACHE = {}
LAST_RUN = {}


def _build(use_bias, use_mask, variant="pe"):
    import concourse.bass as bass
    import concourse.mybir as mybir
    import concourse.tile as tile
    from concourse import bacc

    f32 = mybir.dt.float32
    f32r = mybir.dt.float32r
    bf16 = mybir.dt.bfloat16
    ACT = mybir.ActivationFunctionType
    ALU = mybir.AluOpType

    nc = bacc.Bacc("TRN2", target_bir_lowering=False, debug=False,
                   num_devices=NCORES)

    xT_d = nc.dram_tensor("xT", [C, S], bf16, kind="ExternalInput").ap()
    wqkvT_d = nc.dram_tensor("wqkvT", [C, 3 * LC], bf16, kind="ExternalInput").ap()
    woT_d = nc.dram_tensor("woT", [LC, C], bf16, kind="ExternalInput").ap()
    if use_bias:
        brow_d = nc.dram_tensor("brow", [1, 3 * LC], bf16, kind="ExternalInput").ap()
    if use_mask:
        kmb_d = nc.dram_tensor("kmb", [NKT, KT], f32, kind="ExternalInput").ap()
    out_d = nc.dram_tensor("out", [S, C], bf16, kind="ExternalOutput").ap()

    with tile.TileContext(nc) as tc, \
         nc.allow_low_precision(reason="bf16/f32r matmul inputs are intentionally rounded"):
        # ---------------- persistent SBUF ----------------
        persist = tc.alloc_tile_pool(name="persist", bufs=1)
        # qkT[0..1]: q^T for head pairs (0,1),(2,3); qkT[2..3]: k^T likewise
        qkT = [persist.tile([128, S], f32r, tag=f"qkT{m}", name=f"qkT{m}")
               for m in range(4)]
        # V with an appended ones column per head: [128, kt, head, HD+1]
        V_sb = persist.tile([128, NKT, LH, HD + 1], bf16, tag="V", name="V")
        attnT = [persist.tile([128, S], bf16, tag=f"attnT{i}", name=f"attnT{i}")
                 for i in range(2)]
        woT_sb = [persist.tile([128, C], bf16, tag=f"woT{i}", name=f"woT{i}")
                  for i in range(2)]
        ones_f = persist.tile([128, 64], f32, tag="ones_f", name="ones_f")
        ones_r = persist.tile([1, 64], f32r, tag="ones_r", name="ones_r")

        nc.vector.memset(ones_f[:], 1.0)
        nc.vector.tensor_copy(ones_r[:], ones_f[0:1, 0:64])
        if variant.startswith("gps"):
            # causal triangle for diagonal 128-blocks: 1.0 where q >= k.
            # Built once via affine_select; afterwards gpsimd runs ONLY
            # partition_broadcast (mixing op types costs ~5us ucode reloads).
            tri = persist.tile([128, 2, KT], bf16, tag="tri", name="tri")
            nc.vector.memset(tri[:], 1.0)
            nc.gpsimd.affine_select(
                out=tri[:], in_=tri[:], compare_op=mybir.AluOpType.is_ge,
                fill=0.0, base=0, pattern=[[0, 2], [1, KT]],
                channel_multiplier=-1)
        # ones columns of V (never touched by the V copies below)
        nc.vector.tensor_copy(V_sb[:, :, :, HD], ones_f[:, 0:NKT * LH].rearrange(
            "p (k h) -> p k h", k=NKT))
        if use_bias:
            brow_sb = persist.tile([1, 3 * LC], bf16, tag="brow", name="brow")
            ones_row = persist.tile([1, S], bf16, tag="ones_row", name="ones_row")
            nc.sync.dma_start(out=brow_sb[:], in_=brow_d[:])
            big1 = persist.tile([1, S], f32, tag="big1", name="big1")
            nc.vector.memset(big1[:], 1.0)
            nc.vector.tensor_copy(ones_row[:], big1[:])
        if use_mask:
            kmb_sb = persist.tile([128, NKT], f32, tag="kmb", name="kmb")
            for t in range(NKT):
                nc.sync.dma_start(out=kmb_sb[:, t:t + 1],
                                  in_=kmb_d[t, :].unsqueeze(1))

        wpool = tc.alloc_tile_pool(name="wq", bufs=1)
        xpool = tc.alloc_tile_pool(name="xt", bufs=NSC)
        pt_pool = tc.alloc_tile_pool(name="pt", bufs=6)
        nrm_pool = tc.alloc_tile_pool(name="nrm", bufs=6)
        out_pool = tc.alloc_tile_pool(name="outp", bufs=4)
        # PSUM: 8 banks total: s_ps 2x2 + av_ps 2x1 + g_ps 2x1
        s_ps = tc.alloc_tile_pool(name="ps_s", bufs=2, space="PSUM")
        av_ps = tc.alloc_tile_pool(name="ps_av", bufs=2, space="PSUM")
        g_ps = tc.alloc_tile_pool(name="ps_g", bufs=2, space="PSUM")

        # input DMAs. W and x-chunk-0 stream per 128-row c-tile, interleaved
        # (w_c, x_c) in HWDGE FIFO order, so chunk 0's c-outer QKV can start
        # on tile c as soon as its two DMAs land; later x chunks are one DMA
        # each and drain during chunk-0 compute.
        w_tiles, x0_tiles = [], []
        for c in range(NCT):
            wt = wpool.tile([128, 3 * LC], bf16, tag=f"wc{c}", name=f"wc{c}",
                            bufs=1)
            nc.sync.dma_start(out=wt[:],
                              in_=wqkvT_d[c * 128:(c + 1) * 128, :])
            w_tiles.append(wt)
            xt = xpool.tile([128, SC], bf16, tag=f"x0c{c}", name=f"x0c{c}",
                            bufs=1)
            nc.sync.dma_start(out=xt[:],
                              in_=xT_d[c * 128:(c + 1) * 128, 0:SC])
            x0_tiles.append(xt)
        x_big = {}
        for sc in range(1, NSC):
            xt = xpool.tile([128, NCT, SC], bf16, tag="xt", name="xt")
            if sc == 1:
                # split into halves: the first half gates chunk-0's QKV(1)
                # fillers and lands ~1.5us earlier than the full megabyte
                for h in range(2):
                    nc.sync.dma_start(
                        out=xt[:, 4 * h:4 * (h + 1), :],
                        in_=xT_d[512 * h:512 * (h + 1),
                                 sc * SC:(sc + 1) * SC].rearrange(
                            "(c p) f -> p c f", p=128))
            else:
                nc.sync.dma_start(
                    out=xt[:],
                    in_=xT_d[:, sc * SC:(sc + 1) * SC].rearrange(
                        "(c p) f -> p c f", p=128))
            x_big[sc] = xt
        for i in range(2):
            nc.sync.dma_start(out=woT_sb[i][:],
                              in_=woT_d[i * 128:(i + 1) * 128, :])

        def w_at(c, f0, f1):
            return w_tiles[c][:, f0:f1]

        def x_at(sc, c, f0=0, f1=SC):
            if sc == 0:
                return x0_tiles[c][:, f0:f1]
            return x_big[sc][:, c, f0:f1]

        # ---------------- filler emitters ----------------
        # Each filler quantum emits ~1 matmul (or the trailing copy of a
        # group). Interleaved into the attention PE stream to fill exp-latency
        # gaps. All accumulating matmuls use skip_group_check because
        # unrelated matmuls interleave within their psum accumulation groups.
        def qkv_fillers(sc, ms=(0, 1, 2, 3), sts=(0, 1, 2, 3)):
            """QKV projection of s-chunk sc -> qkT[m][:, sc*SC:], V_sb[:, 4sc:4sc+4]."""
            fillers = []
            st8 = {}

            def qk_mm(m, c):
                if c == 0:
                    st8['qk'] = g_ps.tile([128, SC], f32, tag="g", name="g")
                ps = st8['qk']
                nc.tensor.matmul(ps[:], w_at(c, m * 128, (m + 1) * 128),
                                 x_at(sc, c),
                                 start=(c == 0),
                                 stop=(c == NCT - 1 and not use_bias),
                                 skip_group_check=True)
                if c == NCT - 1 and use_bias:
                    nc.tensor.matmul(ps[:], brow_sb[:, m * 128:(m + 1) * 128],
                                     ones_row[:, sc * SC:(sc + 1) * SC],
                                     start=False, stop=True,
                                     skip_group_check=True)

            def qk_copy(m):
                nc.vector.tensor_copy(qkT[m][:, sc * SC:(sc + 1) * SC],
                                      st8['qk'][:])
                st8['qk'] = None

            def v_mm(st, c):
                if c == 0:
                    st8['v'] = g_ps.tile([128, SC], f32, tag="g", name="g")
                ps = st8['v']
                nc.tensor.matmul(ps[0:128, 0:LC],
                                 x_at(sc, c, st * 128, (st + 1) * 128),
                                 w_at(c, 2 * LC, 3 * LC),
                                 start=(c == 0),
                                 stop=(c == NCT - 1 and not use_bias),
                                 skip_group_check=True)
                if c == NCT - 1 and use_bias:
                    nc.tensor.matmul(
                        ps[0:128, 0:LC],
                        ones_row[:, sc * SC + st * 128:sc * SC + (st + 1) * 128],
                        brow_sb[:, 2 * LC:3 * LC], start=False, stop=True,
                        skip_group_check=True)

            def v_copy(st):
                nc.vector.tensor_copy(
                    V_sb[:, sc * 4 + st, :, 0:HD],
                    st8['v'][0:128, 0:LC].rearrange("p (h d) -> p h d", h=LH))
                st8['v'] = None

            # interleave the long qk matmuls (N=512) with the short v
            # matmuls (N=256): each v LDWEIGHTS (107ns, same size as its
            # matmul) hides under the preceding qk matmul's stream instead
            # of being exposed. Two g_ps tiles (one per group) are live at
            # once -- exactly the pool's 2 bufs.
            for k in range(max(len(ms), len(sts))):
                for c in range(NCT):
                    if k < len(ms):
                        fillers.append(lambda m=ms[k], c=c: qk_mm(m, c))
                    if k < len(sts):
                        fillers.append(lambda st=sts[k], c=c: v_mm(st, c))
                if k < len(ms):
                    fillers.append(lambda m=ms[k]: qk_copy(m))
                if k < len(sts):
                    fillers.append(lambda st=sts[k]: v_copy(st))
            return fillers

        def outproj_fillers(qc):
            """Out-projection of q-chunk qc: attnT[:, st*KT:] @ woT -> out_d."""
            fillers = []
            st8 = {}

            def op_mm(st, co, ci):
                if ci == 0:
                    st8['ps'] = g_ps.tile([128, SC], f32, tag="g", name="g")
                nc.tensor.matmul(st8['ps'][:],
                                 attnT[ci][:, st * KT:(st + 1) * KT],
                                 woT_sb[ci][:, co * SC:(co + 1) * SC],
                                 start=(ci == 0), stop=(ci == 1),
                                 skip_group_check=True)

            def op_copy(st, co):
                o = out_pool.tile([128, SC], bf16, tag="o", name="o")
                nc.vector.tensor_copy(o[:], st8['ps'][:])
                st8['ps'] = None
                nc.sync.dma_start(
                    out=out_d[st * KT:(st + 1) * KT, co * SC:(co + 1) * SC],
                    in_=o[:])

            for st in range(4 * qc, 4 * qc + 4):
                for co in range(2):
                    for ci in range(2):
                        fillers.append(lambda st=st, co=co, ci=ci: op_mm(st, co, ci))
                    fillers.append(lambda st=st, co=co: op_copy(st, co))
            return fillers

        # ---------------- fused pipeline ----------------
        # HAM warmup: tiny matmuls on the ones tiles while the first input
        # DMAs land. They keep the PE activity monitor busy so the real
        # QKV stream starts at the 2.4 GHz clock instead of cold 1.2 GHz.
        warm = g_ps.tile([128, SC], f32, tag="g", name="g")
        for _ in range(44):
            nc.tensor.matmul(warm[0:64, 0:64], ones_r[:], ones_r[:],
                             start=True, stop=True, skip_group_check=True)
        warm = None

        # Chunk 0's QKV runs up front, c-outer so the PE consumes w/x tiles
        # as their DMAs land (all 4 qk groups + 4 V groups accumulate in
        # parallel across the 6 banks attention isn't using yet).
        qk0 = [s_ps.tile([128, 2, SC], f32, tag="ps_s", name="ps_s")
               for _ in range(2)]
        # one accumulation group per psum bank
        v0 = [g_ps.tile([128, SC], f32, tag="g", name="g") for _ in range(2)] \
            + [av_ps.tile([128, SC], f32, tag="ps_av", name="ps_av")
               for _ in range(2)]
        for c in range(NCT):
            last = (c == NCT - 1 and not use_bias)
            for g in range(4):
                # interleave long qk (N=512) with short v (N=256) matmuls so
                # every v LDWEIGHTS hides under a qk stream
                nc.tensor.matmul(qk0[g // 2][:, g % 2, :],
                                 w_at(c, g * 128, (g + 1) * 128), x_at(0, c),
                                 start=(c == 0), stop=last,
                                 skip_group_check=True)
                nc.tensor.matmul(
                    v0[g][0:128, 0:LC],
                    x_at(0, c, g * 128, (g + 1) * 128),
                    w_at(c, 2 * LC, 3 * LC),
                    start=(c == 0), stop=last, skip_group_check=True)
        if use_bias:
            for m in range(4):
                nc.tensor.matmul(qk0[m // 2][:, m % 2, :],
                                 brow_sb[:, m * 128:(m + 1) * 128],
                                 ones_row[:, 0:SC], start=False, stop=True,
                                 skip_group_check=True)
            for st in range(4):
                nc.tensor.matmul(
                    v0[st][0:128, 0:LC],
                    ones_row[:, st * 128:(st + 1) * 128],
                    brow_sb[:, 2 * LC:3 * LC], start=False, stop=True,
                    skip_group_check=True)
        for m in range(4):
            nc.vector.tensor_copy(qkT[m][:, 0:SC], qk0[m // 2][:, m % 2, :])
        for st in range(4):
            nc.vector.tensor_copy(
                V_sb[:, st, :, 0:HD],
                v0[st][0:128, 0:LC].rearrange("p (h d) -> p h d", h=LH))
        qk0 = v0 = None

        norm_tail = []                # deferred normalize phase-B closures
        for qc in range(NSC):
            q0 = qc * SC
            T = 4 * (qc + 1)          # k tiles this q-chunk attends to
            # Chunks 0..2 are PE-bound (attention + just-in-time QKV of the
            # next chunk); chunk 3 is ACT-bound (longest exp stream, no QKV
            # left), so all deferred out-projections go there as PE fillers.
            fillers = []
            if qc + 1 < NSC:
                fillers += qkv_fillers(qc + 1)
            else:
                for pq in range(NSC - 1):
                    fillers += outproj_fillers(pq)
            fq = iter(fillers)
            nfill = len(fillers)
            # spread fillers evenly over the 2*T attention tiles
            per_tile = [nfill // (2 * T) + (1 if k < nfill % (2 * T) else 0)
                        for k in range(2 * T)]

            for p in range(2):        # head pair (2p, 2p+1), PE-packed
                # extra fillers at the pair switch keep the PE busy while the
                # previous pair's normalize chain frees its psum banks
                for _ in range(5 if variant == "gps5" else 3):
                    try:
                        next(fq)()
                    except StopIteration:
                        break
                # phase B of the previous pair's normalize (PE broadcast +
                # mult) runs here, after the switch fillers, so it is never
                # head-of-line for the PE while its DVE inputs are pending
                for fn in norm_tail:
                    fn()
                norm_tail.clear()
                ps_o = [av_ps.tile([HD + 1, SC], f32, tag="ps_av",
                                   name="ps_av") for _ in range(2)]
                prev = None           # software pipeline: AV lags scores by 1
                for t in range(T):
                    kt0 = t * KT
                    diag = kt0 >= q0
                    c0 = (kt0 - q0) if diag else 0   # first valid q column
                    ps_s = s_ps.tile([128, 2, SC], f32, tag="ps_s", name="ps_s")
                    for i in range(2):
                        nc.tensor.matmul(
                            ps_s[:, i, c0:SC],
                            qkT[2 + p][i * 64:(i + 1) * 64, kt0:kt0 + KT],
                            qkT[p][i * 64:(i + 1) * 64, q0 + c0:q0 + SC],
                            start=True, stop=True, tile_position=(i * 64, 0))
                    pt = pt_pool.tile([128, 2, SC], bf16, tag="pt", name="pt")
                    if use_mask:
                        for i in range(2):
                            nc.scalar.activation(
                                pt[:, i, c0:SC], ps_s[:, i, c0:SC], ACT.Exp,
                                bias=kmb_sb[:, t:t + 1], scale=1.0)
                    else:
                        nc.scalar.activation(pt[:, :, c0:SC], ps_s[:, :, c0:SC],
                                             ACT.Exp, bias=0.0, scale=1.0)
                    if diag:
                        # keep q >= k inside the 128-wide boundary block
                        if variant.startswith("gps"):
                            nc.vector.tensor_tensor(
                                pt[:, :, c0:c0 + KT], pt[:, :, c0:c0 + KT],
                                tri[:], ALU.mult)
                        else:
                            nc.gpsimd.affine_select(
                                out=pt[:, :, c0:c0 + KT],
                                in_=pt[:, :, c0:c0 + KT],
                                compare_op=mybir.AluOpType.is_ge, fill=0.0,
                                base=0, pattern=[[0, 2], [1, KT]],
                                channel_multiplier=-1)
                    for _ in range(per_tile[p * T + t]):
                        try:
                            next(fq)()
                        except StopIteration:
                            break
                    if prev is not None:
                        pv_t, pv_pt, pv_c0 = prev
                        for i in range(2):
                            nc.tensor.matmul(
                                ps_o[i][:, pv_c0:SC],
                                V_sb[:, pv_t, 2 * p + i, :],
                                pv_pt[:, i, pv_c0:SC],
                                start=(pv_t == 0), stop=False,
                                skip_group_check=True)
                    prev = (t, pt, c0)
                # drain the software pipeline: AV of the last tile
                pv_t, pv_pt, pv_c0 = prev
                for i in range(2):
                    nc.tensor.matmul(ps_o[i][:, pv_c0:SC],
                                     V_sb[:, pv_t, 2 * p + i, :],
                                     pv_pt[:, i, pv_c0:SC],
                                     start=(pv_t == 0), stop=True,
                                     skip_group_check=True)
                # normalize rows 0..HD-1 by row HD (the ones-column sums).
                # Phase A (now, all DVE): stage the AV psum to SBUF so the av
                # bank frees after ~1us -- the next pair's first AV
                # (start=True) reuses this bank and must not wait for the
                # full normalize chain. The denominator goes to a partition-0
                # tile (the custom-DVE fast reciprocal mis-reads nonzero
                # base partitions on HW), then fast-recip + f32r round.
                # Phase B (deferred into the next pair's filler window):
                # PE-broadcast the reciprocal row, copy, mult into attnT --
                # only the deferred out-projections in the last chunk need
                # the result. (gpsimd partition_broadcast would avoid the PE
                # matmul but swaps the Q7 ucode library against
                # affine_select, a hidden ~5us stall per swap.)
                # the very last pair: no later pair reuses the av banks, so
                # skip the staging copy (mult reads psum directly) and run
                # phase B inline after a burst of warm matmuls that bridges
                # the PE through the normalize chain -- otherwise HAM
                # re-throttles and the whole final out-projection runs at
                # the cold 1.2 GHz clock.
                last_pair = (qc == NSC - 1 and p == 1)
                for i in range(2):
                    if last_pair:
                        stage = None
                    else:
                        stage = nrm_pool.tile([HD, SC], f32, tag="avst",
                                              name="avst", bufs=4)
                        nc.vector.tensor_copy(stage[:], ps_o[i][0:HD, :])
                    dtmp = nrm_pool.tile([1, SC], f32, tag="dtmp", name="dtmp")
                    nc.vector.tensor_copy(dtmp[:], ps_o[i][HD:HD + 1, :])
                    rtmp = nrm_pool.tile([1, SC], f32, tag="rtmp", name="rtmp")
                    nc.vector.reciprocal_approx_fast(rtmp[:], dtmp[:])
                    if variant.startswith("gps"):
                        recip = None
                    else:
                        recip = nrm_pool.tile([1, SC], f32r, tag="recip",
                                              name="recip")
                        nc.vector.tensor_copy(recip[:], rtmp[:])

                    def phase_b(p=p, i=i, q0=q0, ps_o=ps_o, stage=stage,
                                recip=recip, rtmp=rtmp):
                        rb = nrm_pool.tile([64, SC], f32, tag="rb", name="rb")
                        if variant.startswith("gps"):
                            nc.gpsimd.partition_broadcast(rb[:], rtmp[:])
                        else:
                            ps_b = g_ps.tile([128, SC], f32, tag="g", name="g")
                            nc.tensor.matmul(ps_b[0:64, :], ones_r[:],
                                             recip[:], start=True, stop=True,
                                             skip_group_check=True)
                            nc.vector.tensor_copy(rb[:], ps_b[0:64, :])
                        nc.vector.tensor_tensor(
                            attnT[p][i * 64:(i + 1) * 64, q0:q0 + SC],
                            stage[:] if stage is not None else ps_o[i][0:HD, :],
                            rb[:], ALU.mult)
                    norm_tail.append(phase_b)
                if last_pair:
                    warm2 = g_ps.tile([128, SC], f32, tag="g", name="g")
                    for _ in range(64):
                        nc.tensor.matmul(warm2[0:64, 0:64], ones_r[:],
                                         ones_r[:], start=True, stop=True,
                                         skip_group_check=True)
                    warm2 = None
                    for fn in norm_tail:
                        fn()
                    norm_tail.clear()
            # leftover fillers of this chunk (shouldn't normally trigger)
            for f in fq:
                f()

        # last pair's normalize phase B, then the final chunk's out-projection
        for fn in norm_tail:
            fn()
        norm_tail.clear()
        for f in outproj_fillers(NSC - 1):
            f()

        for pool in (g_ps, av_ps, s_ps, out_pool, nrm_pool, pt_pool,
                     xpool, wpool, persist):
            pool.release()

    nc.compile()
    return nc


def _in_maps(x, mask, Wqkv_w, Wqkv_b, Wo_w, Wo_b, use_bias, use_mask):
    bf16 = ml_dtypes.bfloat16
    xT = [np.ascontiguousarray(x[b].T).astype(bf16) for b in range(B)]
    maps = []
    for core in range(NCORES):
        b, hg = core // GROUPS, core % GROUPS
        r = slice(hg * LC, (hg + 1) * LC)
        w_local = np.concatenate([Wqkv_w[r] * np.float32(1.0 / np.sqrt(HD)),
                                  Wqkv_w[C + r.start:C + r.stop],
                                  Wqkv_w[2 * C + r.start:2 * C + r.stop]], axis=0)
        m = {
            "xT": xT[b],
            "wqkvT": np.ascontiguousarray(w_local.T).astype(bf16),
            "woT": np.ascontiguousarray(Wo_w[:, r].T).astype(bf16),
        }
        if use_bias:
            b_local = np.concatenate([Wqkv_b[r] * np.float32(1.0 / np.sqrt(HD)),
                                      Wqkv_b[C + r.start:C + r.stop],
                                      Wqkv_b[2 * C + r.start:2 * C + r.stop]])
            m["brow"] = np.ascontiguousarray(b_local[None, :]).astype(bf16)
        if use_mask:
            m["kmb"] = np.where(mask[b], np.float32(-1e30),
                                np.float32(0.0)).reshape(NKT, KT)
        maps.append(m)
    return maps


def kernel(x, mask, Wqkv_w, Wqkv_b, Wo_w, Wo_b):
    from concourse.bass_utils import run_bass_kernel_spmd

    x = np.asarray(x, dtype=np.float32)
    mask = np.asarray(mask)
    Wqkv_w = np.asarray(Wqkv_w, dtype=np.float32)
    Wqkv_b = np.asarray(Wqkv_b, dtype=np.float32)
    Wo_w = np.asarray(Wo_w, dtype=np.float32)
    Wo_b = np.asarray(Wo_b, dtype=np.float32)

    use_bias = bool(np.any(Wqkv_b))
    use_mask = bool(np.any(mask))
    key = (use_bias, use_mask)
    if key not in _CACHE:
        _CACHE[key] = _build(use_bias, use_mask, variant="gps")
    nc = _CACHE[key]

    maps = _in_maps(x, mask, Wqkv_w, Wqkv_b, Wo_w, Wo_b, use_bias, use_mask)
    res = run_bass_kernel_spmd(nc, maps, list(range(NCORES)))
    LAST_RUN.clear()
    LAST_RUN.update(exec_time_ns=res.exec_time_ns,
                    mean_exec_time_ns=res.mean_exec_time_ns)

    out = np.empty((B, S, C), dtype=np.float32)
    for b in range(B):
        acc = np.zeros((S, C), dtype=np.float64)
        for hg in range(GROUPS):
            acc += res.results[b * GROUPS + hg]["out"].astype(np.float64)
        out[b] = (acc + Wo_b.astype(np.float64)).astype(np.float32)
    return out

